# revision 40
# baseline (speedup 1.0000x reference)
"""Self-contained Trainium2 kernel for nn_Attention_24799141167815.

Cosine-similarity attention (Swin-v2 style) with continuous position bias.
Data-parallel over batch B=8 across 8 NeuronCores (core b handles batch b).

Device math per core (batch b):
  A:  qkT = wqkT.T @ xT            (raw q|k feats, feat-major [128, fb, tok])
      v   = xT.T @ wvT             (tok-major [tok, h, 128]: col 0 = ones ->
                                    softmax denominator; v at cols 64:128;
                                    v-bias folded into proj bias on host)
      sq  = qkT*qkT inline; per-head sumsq via scatter-stationary matmuls
      into ssq[8,tok] / ssk[8,tok]; the q-norm chain (recip_approx_fast,
      clamp, ACT-sqrt with scale_h^2 folded, DRAM-round-trip broadcast to
      rnb, qsT = qkT_q*rnb) fires mid-stage-A.
  B:  rkT = PE-transpose of k-norms -> [tok-part, kt, h] (f32, ACT scale)
      qekn = (qkT_k.T @ qesbd) * rkT      per-key query-embedding bias
  D:  per (h, kt): s_raw = qkT_k_h.T @ qsT_h           ([128 key, 1024 q])
      etmp = exp(s_raw * rkT + qekn)     (ACT: k-norm + bias fused via
                                          per-partition scale/bias)
      eT   = etmp * expb_h               (DVE only; GpSimd is 3x slower and
                                          its latency bubbles the PE queue)
      pav += v_h.T @ eT                  ([128, 1024]; row 0 = denominator)
      AV matmuls trail the muls by 2 kt; each head's tail AVs + division
      prep are deferred into the next head's QK stream; the division mul
      (pav[64:128] * rrb) is deferred one further head so the rrb DMA
      round-trip (rrec -> DRAM -> 64-partition broadcast) never stalls DVE.
      expb_h prefetched 3 heads ahead as one 2MB DMA on the sync ring.
  E:  outT = projwT.T @ outhT + projb'   (c-major [512, 1024]; host .T)
"""

import os
import numpy as np
import ml_dtypes

import concourse.bass as bass
import concourse.mybir as mybir
import concourse.tile as tile
from concourse import bacc
from concourse.bass_utils import run_bass_kernel_spmd

F32 = mybir.dt.float32
BF16 = mybir.dt.bfloat16
AF = mybir.ActivationFunctionType

B, N, C = 8, 1024, 512
H, HD = 8, 64
NT = N // 128     # 8 token/key tiles
CB = C // 128     # 4 cin blocks
FB = 2 * CB       # 8 q+k feature blocks
NB_BF16 = np.dtype(ml_dtypes.bfloat16)

_CACHE = {}


def _build(stage=4):
    nc = bacc.Bacc("TRN2", target_bir_lowering=False)

    xT_d = nc.declare_dram_parameter("xT", [C, N], BF16, isOutput=False)
    wqkT_d = nc.declare_dram_parameter("wqkT", [C, 2 * C], BF16, isOutput=False)
    wvT_d = nc.declare_dram_parameter("wvT", [C, C], BF16, isOutput=False)
    qkb_d = nc.declare_dram_parameter("qkb", [2 * C, 1], F32, isOutput=False)
    qesbd_d = nc.declare_dram_parameter("qesbd", [C, H], BF16, isOutput=False)
    scalesq_d = nc.declare_dram_parameter("scalesq", [8, 1], F32, isOutput=False)
    projwT_d = nc.declare_dram_parameter("projwT", [C, C], BF16, isOutput=False)
    projb_d = nc.declare_dram_parameter("projb", [C, 1], F32, isOutput=False)
    ssum16_d = nc.declare_dram_parameter("ssum16", [2 * C, 8], BF16, isOutput=False)
    ident_d = nc.declare_dram_parameter("ident", [128, 128], BF16, isOutput=False)
    expbT_d = nc.declare_dram_parameter("expbT", [H, 128, NT * N], BF16, isOutput=False)
    outT_d = nc.declare_dram_parameter("outT", [C, N], F32, isOutput=True)

    with tile.TileContext(nc) as tc:
        with (
            tc.tile_pool(name="persist", bufs=1) as persist,
            tc.tile_pool(name="expbp", bufs=3) as expbp,
            tc.tile_pool(name="etp", bufs=4) as etp,
            tc.tile_pool(name="eTp", bufs=8) as eTp,
            tc.tile_pool(name="sqp", bufs=3) as sqp,
            tc.tile_pool(name="rrp", bufs=3) as rrp,
            tc.tile_pool(name="osp", bufs=2) as osp,
            tc.tile_pool(name="dram", bufs=3, space="DRAM") as dram,
            tc.tile_pool(name="ps_qk", bufs=2, space="PSUM") as ps_qk,
            tc.tile_pool(name="ps_av", bufs=2, space="PSUM") as ps_av,
        ):
            # -------- load constants / weights (sync ring, ordered) --------
            # order matters: the HWDGE ring is in-order; stage-A deps first,
            # then early expb prefetches interleaved with later-needed params.
            xT = persist.tile([128, CB, N], BF16, tag="xT")
            wqkT = persist.tile([128, CB, 2 * C], BF16, tag="wqkT")
            xT_src = xT_d.rearrange("(cb p) n -> p cb n", p=128)
            wqk_src = wqkT_d.rearrange("(cb p) f -> p cb f", p=128)
            for cb in range(CB):
                nc.sync.dma_start(out=xT[:, cb, :], in_=xT_src[:, cb, :])
            qkb = persist.tile([128, FB], F32, tag="qkb")
            nc.sync.dma_start(
                out=qkb, in_=qkb_d.rearrange("(fb p) one -> p (fb one)", p=128))
            for fb in range(FB):
                nc.sync.dma_start(
                    out=wqkT[:, :, fb * 128:(fb + 1) * 128],
                    in_=wqk_src[:, :, fb * 128:(fb + 1) * 128])
            expb_tiles = [expbp.tile([128, NT, N], BF16, tag="expb",
                                     name=f"expb{i}")
                          for i in range(min(3, H))] if stage >= 3 else []
            if stage >= 3:
                nc.sync.dma_start(
                    out=expb_tiles[0],
                    in_=expbT_d[0].rearrange("p (kt q) -> p kt q", kt=NT))
            wvT = persist.tile([128, CB, C], BF16, tag="wvT")
            nc.sync.dma_start(out=wvT, in_=wvT_d.rearrange("(cb p) f -> p cb f", p=128))
            qesbd = persist.tile([128, CB, H], BF16, tag="qesbd")
            nc.sync.dma_start(
                out=qesbd, in_=qesbd_d.rearrange("(cb p) h -> p cb h", p=128))
            ssum16 = persist.tile([128, FB, 8], BF16, tag="ssum16")
            nc.sync.dma_start(
                out=ssum16, in_=ssum16_d.rearrange("(fb p) r -> p fb r", p=128))
            scalesq = persist.tile([8, 1], F32, tag="scalesq")
            nc.sync.dma_start(out=scalesq, in_=scalesq_d[:])
            ident = persist.tile([128, 128], BF16, tag="ident")
            nc.sync.dma_start(out=ident, in_=ident_d[:])
            if stage >= 3:
                nc.sync.dma_start(
                    out=expb_tiles[1],
                    in_=expbT_d[1].rearrange("p (kt q) -> p kt q", kt=NT))
            projwT = persist.tile([128, CB, C], BF16, tag="projwT")
            nc.sync.dma_start(
                out=projwT, in_=projwT_d.rearrange("(cb p) f -> p cb f", p=128))
            projb = persist.tile([128, CB], F32, tag="projb")
            nc.sync.dma_start(
                out=projb, in_=projb_d.rearrange("(cb p) one -> p (cb one)", p=128))
            if stage >= 3:
                nc.sync.dma_start(
                    out=expb_tiles[2],
                    in_=expbT_d[2].rearrange("p (kt q) -> p kt q", kt=NT))

            # v[tok, h, 128] tok-major; col 0 = ones (denominator -> pav
            # row 0: base-0 input for the custom-DVE recip), v at cols 64:128
            # (pav rows 64:128 -- 64-partition windows must start at 0 or 64)
            v_sb = persist.tile([128, NT, H, 128], BF16, tag="v_sb")
            nc.vector.memset(v_sb[:, :, :, 0:64], 0.0)
            nc.vector.memset(v_sb[:, :, :, 0:1], 1.0)
            # qkT[feat, tok] raw q+k features, with q/k bias added on copy
            qkT = persist.tile([128, FB, N], BF16, tag="qkT")
            qsT = persist.tile([128, CB, N], BF16, tag="qsT")
            rkT = persist.tile([128, NT, H], F32, tag="rkT")
            qekn = persist.tile([128, NT, H], F32, tag="qekn")
            for fb in range(FB):
                ps = ps_qk.tile([128, N], F32, tag="qk")
                for qb in range(2):
                    for cb in range(CB):
                        nc.tensor.matmul(
                            ps[:, qb * 512:(qb + 1) * 512],
                            wqkT[:, cb, fb * 128:(fb + 1) * 128],
                            xT[:, cb, qb * 512:(qb + 1) * 512],
                            start=(cb == 0), stop=(cb == CB - 1),
                        )
                # psum -> sbuf with per-partition bias add: q-half on DVE
                # (feeds the sumsq chain), k-half on the idle-in-A ACT
                if fb < CB:
                    nc.vector.tensor_scalar_add(qkT[:, fb, :], ps, qkb[:, fb:fb + 1])
                else:
                    nc.scalar.activation(
                        out=qkT[:, fb, :], in_=ps, func=AF.Identity,
                        bias=qkb[:, fb:fb + 1], scale=1.0)
                # v-projection tile tb==fb interleaved: keeps the PE dense
                # through stage A and clears the stage-B bubble
                vps = ps_av.tile([128, C], F32, tag="av", name=f"vps{fb}")
                for cb in range(CB):
                    nc.tensor.matmul(
                        vps,
                        xT[:, cb, fb * 128:(fb + 1) * 128],
                        wvT[:, cb, :],
                        start=(cb == 0), stop=(cb == CB - 1),
                    )
                nc.vector.tensor_copy(
                    v_sb[:, fb, :, 64:128], vps.rearrange("p (h d) -> p h d", h=H))
                if stage >= 2:
                    sq = sqp.tile([128, N], BF16, tag="sq", name=f"sq{fb}")
                    nc.vector.tensor_mul(sq, qkT[:, fb, :], qkT[:, fb, :])
                    if fb == 0:
                        ssq = ps_av.tile([8, N], F32, tag="av", name="ssq")
                    if fb == CB:
                        ssk = ps_av.tile([8, N], F32, tag="av", name="ssk")
                    tgt = ssq if fb < CB else ssk
                    for qb in range(2):
                        nc.tensor.matmul(
                            tgt[:, qb * 512:(qb + 1) * 512],
                            ssum16[:, fb, :],
                            sq[:, qb * 512:(qb + 1) * 512],
                            start=(fb % CB == 0), stop=(fb % CB == CB - 1),
                        )
                    if fb == CB - 1:
                        # early q-norm chain: fires while the k-half of
                        # stage A is still on the PE
                        rivq = persist.tile([8, N], F32, tag="rivq")
                        nc.vector.reciprocal_approx_fast(rivq, ssq)
                        nc.vector.tensor_scalar_min(rivq, rivq, 1e24)
                        rnormq = persist.tile([8, N], BF16, tag="rnormq")
                        nc.scalar.activation(
                            out=rnormq, in_=rivq, func=AF.Sqrt, bias=0.0,
                            scale=scalesq[:, 0:1])
                        rnq_d = dram.tile([8, N], BF16, tag="rnq_d")
                        nc.gpsimd.dma_start(out=rnq_d, in_=rnormq)
                        rnb = persist.tile([128, CB, N], BF16, tag="rnb")
                        for f in range(CB):
                            for phi in range(2):
                                nc.gpsimd.dma_start(
                                    out=rnb[64 * phi:64 * (phi + 1), f, :],
                                    in_=rnq_d[2 * f + phi:2 * f + phi + 1, :]
                                    .to_broadcast((64, N)))
                        for f in range(CB):
                            nc.vector.tensor_mul(
                                qsT[:, f, :], qkT[:, f, :], rnb[:, f, :])
                    if fb == FB - 1:
                        rivk = persist.tile([8, N], F32, tag="rivk")
                        nc.vector.reciprocal_approx_fast(rivk, ssk)
                        nc.vector.tensor_scalar_min(rivk, rivk, 1e24)
                        rnormk = persist.tile([8, N], BF16, tag="rnormk")
                        nc.scalar.activation(
                            out=rnormk, in_=rivk, func=AF.Sqrt, bias=0.0,
                            scale=1.0)

            if stage < 2:
                _debug_out(nc, osp, qkT, outT_d)
            # ---------------- B: k-norm transposes + query-embedding ------
            if stage >= 2:
                # rkT[tok-part, kt, h]: transpose of k rows of rnorm (f32),
                # qekn = (qkT_k.T @ qesbd) * rkT  (query-embedding bias)
                for kt in range(NT):
                    pt = ps_av.tile([128, 8], BF16, tag="av")
                    nc.tensor.transpose(
                        pt, rnormk[0:8, kt * 128:(kt + 1) * 128],
                        ident[0:8, 0:8])
                    nc.vector.tensor_copy(rkT[:, kt, :], pt)
                    pq = ps_av.tile([128, H], F32, tag="av")
                    for cb in range(CB):
                        nc.tensor.matmul(
                            pq,
                            qkT[:, CB + cb, kt * 128:(kt + 1) * 128],
                            qesbd[:, cb, :],
                            start=(cb == 0), stop=(cb == CB - 1),
                        )
                    nc.vector.tensor_mul(qekn[:, kt, :], pq, rkT[:, kt, :])


            if stage == 2:
                _debug_out(nc, osp, qsT, outT_d)
            # ---------------- D: attention ----------------
            outhT = persist.tile([128, CB, N], BF16, tag="outhT")
            pending_div = []
            prev_tail = None

            def flush_div():
                # division-mul deferred a full head: by now the rrb broadcast
                # DMA has landed, so this never stalls the DVE queue head
                ph, prrb, ppav = pending_div.pop(0)
                php, ppo = ph // 2, (ph % 2) * 64
                nc.vector.tensor_mul(
                    outhT[:, php, :][ppo:ppo + 64], ppav[64:128, :], prrb)

            for h in range(H if stage >= 3 else 0):
                hp, po = h // 2, (h % 2) * 64
                expb = expb_tiles[h]
                if h + 3 < H:
                    nxt = expbp.tile([128, NT, N], BF16, tag="expb",
                                     name=f"expb{h + 3}")
                    expb_tiles.append(nxt)
                    nc.sync.dma_start(
                        out=nxt,
                        in_=expbT_d[h + 3].rearrange("p (kt q) -> p kt q", kt=NT))
                pav = ps_av.tile([128, N], F32, tag="av", name=f"pav{h}")

                def av_mm(xpav, xh, kt, eT):
                    for qb in range(2):
                        nc.tensor.matmul(
                            xpav[:, qb * 512:(qb + 1) * 512],
                            v_sb[:, kt, xh, :],
                            eT[:, qb * 512:(qb + 1) * 512],
                            start=(kt == 0), stop=(kt == NT - 1),
                        )

                def finish_head(tail):
                    # previous head's tail AVs + division prep, emitted inside
                    # this head's dense QK stream so nothing bubbles the PE
                    th, tpav, teTs = tail
                    av_mm(tpav, th, NT - 2, teTs[NT - 2])
                    av_mm(tpav, th, NT - 1, teTs[NT - 1])
                    rrec = rrp.tile([1, N], F32, tag="rrec")
                    nc.vector.reciprocal_approx_fast(rrec, tpav[0:1, :])
                    rrec_d = dram.tile([1, N], F32, tag="rrec_d")
                    nc.gpsimd.dma_start(out=rrec_d, in_=rrec)
                    rrb = rrp.tile([64, N], F32, tag="rrb")
                    nc.gpsimd.dma_start(
                        out=rrb, in_=rrec_d[0:1, :].to_broadcast((64, N)))
                    pending_div.append((th, rrb, tpav))
                    if len(pending_div) > 1:
                        flush_div()

                eTs = []
                for kt in range(NT):
                    pss = ps_qk.tile([128, N], F32, tag="qk")
                    for qb in range(2):
                        nc.tensor.matmul(
                            pss[:, qb * 512:(qb + 1) * 512],
                            qkT[:, CB + hp, kt * 128:(kt + 1) * 128][po:po + 64],
                            qsT[:, hp, qb * 512:(qb + 1) * 512][po:po + 64],
                            start=True, stop=True,
                        )
                    etmp = etp.tile([128, N], BF16, tag="etmp")
                    nc.scalar.activation(
                        out=etmp, in_=pss, func=AF.Exp,
                        bias=qekn[:, kt, h:h + 1], scale=rkT[:, kt, h:h + 1])
                    eT = eTp.tile([128, N], BF16, tag="eT", name=f"eT{h}_{kt}")
                    mul_eng = nc.vector
                    mul_eng.tensor_mul(eT, etmp, expb[:, kt, :])
                    eTs.append(eT)
                    if kt == 1 and prev_tail is not None:
                        finish_head(prev_tail)
                    # AV trails the mul by 2 kt so a slow mul cannot bubble
                    # the in-order PE queue
                    if kt >= 2:
                        av_mm(pav, h, kt - 2, eTs[kt - 2])
                prev_tail = (h, pav, eTs)
            if stage >= 3:
                finish_head(prev_tail)
                flush_div()

            if stage == 3:
                _debug_out(nc, osp, outhT, outT_d)
            # ---------------- E: output projection (c-major) ----------------
            # fb0-2 partials of the first two channel groups run during the
            # last head's division round-trip (keeps the PE warm into E)
            if stage >= 4:
                psE = {}

                def e_mms(cc, fbs):
                    for fb in fbs:
                        for th in range(2):
                            nc.tensor.matmul(
                                psE[cc][:, th * 512:(th + 1) * 512],
                                projwT[:, fb, cc * 128:(cc + 1) * 128],
                                outhT[:, fb, th * 512:(th + 1) * 512],
                                start=(fb == 0), stop=(fb == CB - 1),
                            )

                def e_finish(cc):
                    osb = osp.tile([128, N], F32, tag="osb")
                    nc.vector.tensor_scalar_add(osb, psE[cc], projb[:, cc:cc + 1])
                    for half in range(2):
                        eng = nc.scalar if (2 * cc + half) % 2 == 0 else nc.sync
                        eng.dma_start(
                            out=outT_d[cc * 128:(cc + 1) * 128,
                                       half * 512:(half + 1) * 512],
                            in_=osb[:, half * 512:(half + 1) * 512])

                for cc in range(2):
                    psE[cc] = ps_qk.tile([128, N], F32, tag="qk", name=f"psE{cc}")
                    e_mms(cc, [0, 1, 2])
                for cc in range(2):
                    e_mms(cc, [3])
                    e_finish(cc)
                for cc in range(2, CB):
                    psE[cc] = ps_qk.tile([128, N], F32, tag="qk", name=f"psE{cc}")
                    e_mms(cc, [0, 1, 2, 3])
                    e_finish(cc)

    nc.compile()
    return nc


def _debug_out(nc, osp, dbg, outT_d):
    for cc in range(CB):
        osb = osp.tile([128, N], F32, tag="osb")
        nc.vector.tensor_copy(osb, dbg[:, cc, :])
        nc.scalar.dma_start(out=outT_d[cc * 128:(cc + 1) * 128, :], in_=osb)


def _host_prep(inputs):
    """Host-side layout/scalar prep. Returns per-core input maps."""
    x = np.asarray(inputs["x"], dtype=np.float32)
    qkv_w = np.asarray(inputs["qkv_w"], dtype=np.float32)
    qkv_b = np.asarray(inputs["qkv_b"], dtype=np.float32)
    proj_w = np.asarray(inputs["proj_w"], dtype=np.float32)
    proj_b = np.asarray(inputs["proj_b"], dtype=np.float32)
    temp = np.asarray(inputs["temperature"], dtype=np.float32).reshape(H)
    qe = np.asarray(inputs["query_embedding"], dtype=np.float32).reshape(H, HD)
    tab = np.asarray(inputs["relative_coords_table"], dtype=np.float32)
    idx = np.asarray(inputs["relative_pos_index"])
    f1w = np.asarray(inputs["cpb_fc1_w"], dtype=np.float32)
    f1b = np.asarray(inputs["cpb_fc1_b"], dtype=np.float32)
    f2w = np.asarray(inputs["cpb_fc2_w"], dtype=np.float32)
    f2b = np.asarray(inputs["cpb_fc2_b"], dtype=np.float32)
    sls = np.asarray(inputs["seq_length_scale"], dtype=np.float32)

    # softplus(temperature) * seq_length_scale
    scale = (np.logaddexp(0.0, temp) * sls[0]).astype(np.float32)

    # continuous position bias table -> gathered, transposed, exponentiated
    hidden = np.maximum(tab @ f1w.T + f1b, 0.0)
    bias_tab = (hidden @ f2w.T + f2b).astype(np.float32)      # (T, H)
    bias = bias_tab[idx]                                       # (q, k, H)
    expbT = np.exp(np.transpose(bias, (2, 1, 0)))              # (H, k, q)
    # tile so each SBUF partition reads one contiguous 16KB run:
    # [H, kt, p, q] -> [H, p, kt, q]  (k = kt*128 + p)
    expbT = expbT.reshape(H, NT, 128, N).transpose(0, 2, 1, 3)
    expbT = np.ascontiguousarray(expbT).astype(NB_BF16).reshape(H, 128, NT * N)

    wqkT = np.ascontiguousarray(qkv_w[:2 * C].T).astype(NB_BF16)   # (cin, 1024)
    wvT = np.ascontiguousarray(qkv_w[2 * C:].T).astype(NB_BF16)    # (cin, 512)
    projwT = np.ascontiguousarray(proj_w.T).astype(NB_BF16)        # (cin, 512)
    qkb = qkv_b[:2 * C].reshape(2 * C, 1).copy()
    vb = qkv_b[2 * C:]
    # fold v-bias through the projection:  (o + vb) @ W.T + b = o@W.T + b'
    projb = (proj_b + vb @ proj_w.T).reshape(C, 1).astype(np.float32)
    qesbd = np.zeros((C, H), dtype=np.float32)
    for h in range(H):
        qesbd[h * HD:(h + 1) * HD, h] = qe[h] * scale[h]
    qesbd = qesbd.astype(NB_BF16)
    # q-heads get scale_h^2 inside the sqrt (k path uses scale=1.0)
    scalesq = (scale * scale).reshape(8, 1).astype(np.float32)

    # scatter-stationaries for per-head sumsq: [fb][128, 8]
    ssum16 = np.zeros((FB, 128, 8), dtype=NB_BF16)
    for f in range(FB):
        j = f % CB
        ssum16[f, 0:64, 2 * j] = 1.0
        ssum16[f, 64:128, 2 * j + 1] = 1.0
    ssum16 = np.ascontiguousarray(ssum16).reshape(2 * C, 8)
    ident = np.eye(128, dtype=NB_BF16)

    shared = dict(
        wqkT=wqkT, wvT=wvT, qkb=qkb, qesbd=qesbd, scalesq=scalesq,
        projwT=projwT, projb=projb, ssum16=ssum16, ident=ident, expbT=expbT,
    )
    in_maps = []
    for b in range(B):
        m = dict(shared)
        m["xT"] = np.ascontiguousarray(x[b].T).astype(NB_BF16)
        in_maps.append(m)
    return in_maps


def _assemble(res):
    """Gather per-core c-major outputs into the full (B, N, C) result."""
    return np.stack(
        [np.ascontiguousarray(res.results[b]["outT"].T) for b in range(B)],
        axis=0).astype(np.float32)


def get_nc():
    key = "nc"
    if key not in _CACHE:
        stage = int(os.environ.get("BASS_STAGE", "4"))
        _CACHE[key] = _build(stage)
    return _CACHE[key]


def kernel(**inputs) -> np.ndarray:
    nc = get_nc()
    in_maps = _host_prep(inputs)
    res = run_bass_kernel_spmd(nc, in_maps, core_ids=list(range(B)))
    return _assemble(res)


# revision 41
# speedup vs baseline: 1.0565x; 1.0565x over previous
"""Self-contained Trainium2 kernel for nn_Attention_24799141167815.

Cosine-similarity attention (Swin-v2 style) with continuous position bias.
Data-parallel over batch B=8 across 8 NeuronCores (core b handles batch b).

Device math per core (batch b):
  A:  qkT = wqkT.T @ xT            (raw q|k feats, feat-major [128, fb, tok])
      v   = xT.T @ wvT             (tok-major [tok, h, 128]: col 0 = ones ->
                                    softmax denominator; v at cols 64:128;
                                    v-bias folded into proj bias on host)
      sq  = qkT*qkT inline; per-head sumsq via scatter-stationary matmuls
      into ssq[8,tok] / ssk[8,tok]; the q-norm chain (recip_approx_fast,
      clamp, ACT-sqrt with scale_h^2 folded, DRAM-round-trip broadcast to
      rnb, qsT = qkT_q*rnb) fires mid-stage-A.
  B:  rkT = PE-transpose of k-norms -> [tok-part, kt, h] (f32, ACT scale)
      qekn = (qkT_k.T @ qesbd) * rkT      per-key query-embedding bias
  D:  per (h, kt): s_raw = qkT_k_h.T @ qsT_h           ([128 key, 1024 q])
      etmp = exp(s_raw * rkT + qekn)     (ACT: k-norm + bias fused via
                                          per-partition scale/bias)
      eT   = etmp * expb_h               (DVE only; GpSimd is 3x slower and
                                          its latency bubbles the PE queue)
      pav += v_h.T @ eT                  ([128, 1024]; row 0 = denominator)
      AV matmuls trail the muls by 2 kt; each head's tail AVs + division
      prep are deferred into the next head's QK stream; the division mul
      (pav[64:128] * rrb) is deferred one further head so the rrb DMA
      round-trip (rrec -> DRAM -> 64-partition broadcast) never stalls DVE.
      expb_h prefetched 3 heads ahead as one 2MB DMA on the sync ring.
  E:  outT = projwT.T @ outhT + projb'   (c-major [512, 1024]; host .T)
"""

import os
import numpy as np
import ml_dtypes

import concourse.bass as bass
import concourse.mybir as mybir
import concourse.tile as tile
from concourse import bacc
from concourse.bass_utils import run_bass_kernel_spmd

F32 = mybir.dt.float32
BF16 = mybir.dt.bfloat16
AF = mybir.ActivationFunctionType

B, N, C = 8, 1024, 512
H, HD = 8, 64
NT = N // 128     # 8 token/key tiles
CB = C // 128     # 4 cin blocks
FB = 2 * CB       # 8 q+k feature blocks
NB_BF16 = np.dtype(ml_dtypes.bfloat16)

_CACHE = {}


def _build(stage=4):
    nc = bacc.Bacc("TRN2", target_bir_lowering=False)

    xT_d = nc.declare_dram_parameter("xT", [C, N], BF16, isOutput=False)
    wqkT_d = nc.declare_dram_parameter("wqkT", [C, 2 * C], BF16, isOutput=False)
    wvT_d = nc.declare_dram_parameter("wvT", [C, C], BF16, isOutput=False)
    qkb_d = nc.declare_dram_parameter("qkb", [2 * C, 1], F32, isOutput=False)
    qesbd_d = nc.declare_dram_parameter("qesbd", [C, H], BF16, isOutput=False)
    scalesq_d = nc.declare_dram_parameter("scalesq", [8, 1], F32, isOutput=False)
    projwT_d = nc.declare_dram_parameter("projwT", [C, C], BF16, isOutput=False)
    projb_d = nc.declare_dram_parameter("projb", [C, 1], F32, isOutput=False)
    ssum16_d = nc.declare_dram_parameter("ssum16", [2 * C, 8], BF16, isOutput=False)
    ident_d = nc.declare_dram_parameter("ident", [128, 128], BF16, isOutput=False)
    expbT_d = nc.declare_dram_parameter("expbT", [H, 128, NT * N], BF16, isOutput=False)
    outT_d = nc.declare_dram_parameter("outT", [C, N], F32, isOutput=True)

    with tile.TileContext(nc) as tc:
        with (
            tc.tile_pool(name="persist", bufs=1) as persist,
            tc.tile_pool(name="expbp", bufs=3) as expbp,
            tc.tile_pool(name="etp", bufs=4) as etp,
            tc.tile_pool(name="eTp", bufs=8) as eTp,
            tc.tile_pool(name="sqp", bufs=3) as sqp,
            tc.tile_pool(name="rrp", bufs=3) as rrp,
            tc.tile_pool(name="osp", bufs=2) as osp,
            tc.tile_pool(name="dram", bufs=3, space="DRAM") as dram,
            tc.tile_pool(name="ps_qk", bufs=2, space="PSUM") as ps_qk,
            tc.tile_pool(name="ps_av", bufs=2, space="PSUM") as ps_av,
        ):
            # -------- load constants / weights (sync ring, ordered) --------
            # order matters: the HWDGE ring is in-order; stage-A deps first,
            # then early expb prefetches interleaved with later-needed params.
            xT = persist.tile([128, CB, N], BF16, tag="xT")
            wqkT = persist.tile([128, CB, 2 * C], BF16, tag="wqkT")
            xT_src = xT_d.rearrange("(cb p) n -> p cb n", p=128)
            wqk_src = wqkT_d.rearrange("(cb p) f -> p cb f", p=128)
            for cb in range(CB):
                nc.sync.dma_start(out=xT[:, cb, :], in_=xT_src[:, cb, :])
            qkb = persist.tile([128, FB], F32, tag="qkb")
            nc.sync.dma_start(
                out=qkb, in_=qkb_d.rearrange("(fb p) one -> p (fb one)", p=128))
            for fb in range(FB):
                nc.sync.dma_start(
                    out=wqkT[:, :, fb * 128:(fb + 1) * 128],
                    in_=wqk_src[:, :, fb * 128:(fb + 1) * 128])
            expb_tiles = [expbp.tile([128, NT, N], BF16, tag="expb",
                                     name=f"expb{i}")
                          for i in range(min(3, H))] if stage >= 3 else []
            if stage >= 3:
                nc.sync.dma_start(
                    out=expb_tiles[0],
                    in_=expbT_d[0].rearrange("p (kt q) -> p kt q", kt=NT))
            wvT = persist.tile([128, CB, C], BF16, tag="wvT")
            nc.sync.dma_start(out=wvT, in_=wvT_d.rearrange("(cb p) f -> p cb f", p=128))
            qesbd = persist.tile([128, CB, H], BF16, tag="qesbd")
            nc.sync.dma_start(
                out=qesbd, in_=qesbd_d.rearrange("(cb p) h -> p cb h", p=128))
            ssum16 = persist.tile([128, FB, 8], BF16, tag="ssum16")
            nc.sync.dma_start(
                out=ssum16, in_=ssum16_d.rearrange("(fb p) r -> p fb r", p=128))
            scalesq = persist.tile([8, 1], F32, tag="scalesq")
            nc.sync.dma_start(out=scalesq, in_=scalesq_d[:])
            ident = persist.tile([128, 128], BF16, tag="ident")
            nc.sync.dma_start(out=ident, in_=ident_d[:])
            if stage >= 3:
                nc.sync.dma_start(
                    out=expb_tiles[1],
                    in_=expbT_d[1].rearrange("p (kt q) -> p kt q", kt=NT))
            projwT = persist.tile([128, CB, C], BF16, tag="projwT")
            nc.sync.dma_start(
                out=projwT, in_=projwT_d.rearrange("(cb p) f -> p cb f", p=128))
            projb = persist.tile([128, CB], F32, tag="projb")
            nc.sync.dma_start(
                out=projb, in_=projb_d.rearrange("(cb p) one -> p (cb one)", p=128))
            if stage >= 3:
                nc.sync.dma_start(
                    out=expb_tiles[2],
                    in_=expbT_d[2].rearrange("p (kt q) -> p kt q", kt=NT))

            # v[tok, h, 128] tok-major; col 0 = ones (denominator -> pav
            # row 0: base-0 input for the custom-DVE recip), v at cols 64:128
            # (pav rows 64:128 -- 64-partition windows must start at 0 or 64)
            v_sb = persist.tile([128, NT, H, 128], BF16, tag="v_sb")
            nc.vector.memset(v_sb[:, :, :, 0:64], 0.0)
            nc.vector.memset(v_sb[:, :, :, 0:1], 1.0)
            # qkT[feat, tok] raw q+k features, with q/k bias added on copy
            qkT = persist.tile([128, FB, N], BF16, tag="qkT")
            qsT = persist.tile([128, CB, N], BF16, tag="qsT")
            rkT = persist.tile([128, NT, H], F32, tag="rkT")
            qekn = persist.tile([128, NT, H], F32, tag="qekn")
            for fb in range(FB):
                ps = ps_qk.tile([128, N], F32, tag="qk")
                for qb in range(2):
                    for cb in range(CB):
                        nc.tensor.matmul(
                            ps[:, qb * 512:(qb + 1) * 512],
                            wqkT[:, cb, fb * 128:(fb + 1) * 128],
                            xT[:, cb, qb * 512:(qb + 1) * 512],
                            start=(cb == 0), stop=(cb == CB - 1),
                        )
                # psum -> sbuf with per-partition bias add: q-half on DVE
                # (feeds the sumsq chain), k-half on the idle-in-A ACT
                if fb < CB:
                    nc.vector.tensor_scalar_add(qkT[:, fb, :], ps, qkb[:, fb:fb + 1])
                else:
                    nc.scalar.activation(
                        out=qkT[:, fb, :], in_=ps, func=AF.Identity,
                        bias=qkb[:, fb:fb + 1], scale=1.0)
                # v-projection tile tb==fb interleaved: keeps the PE dense
                # through stage A and clears the stage-B bubble
                vps = ps_av.tile([128, C], F32, tag="av", name=f"vps{fb}")
                for cb in range(CB):
                    nc.tensor.matmul(
                        vps,
                        xT[:, cb, fb * 128:(fb + 1) * 128],
                        wvT[:, cb, :],
                        start=(cb == 0), stop=(cb == CB - 1),
                    )
                nc.vector.tensor_copy(
                    v_sb[:, fb, :, 64:128], vps.rearrange("p (h d) -> p h d", h=H))
                if stage >= 2:
                    sq = sqp.tile([128, N], BF16, tag="sq", name=f"sq{fb}")
                    nc.vector.tensor_mul(sq, qkT[:, fb, :], qkT[:, fb, :])
                    if fb == 0:
                        ssq = ps_av.tile([8, N], F32, tag="av", name="ssq")
                    if fb == CB:
                        ssk = ps_av.tile([8, N], F32, tag="av", name="ssk")
                    tgt = ssq if fb < CB else ssk
                    for qb in range(2):
                        nc.tensor.matmul(
                            tgt[:, qb * 512:(qb + 1) * 512],
                            ssum16[:, fb, :],
                            sq[:, qb * 512:(qb + 1) * 512],
                            start=(fb % CB == 0), stop=(fb % CB == CB - 1),
                        )
                    if fb == CB - 1:
                        # early q-norm chain: fires while the k-half of
                        # stage A is still on the PE
                        rivq = persist.tile([8, N], F32, tag="rivq")
                        nc.vector.reciprocal_approx_fast(rivq, ssq)
                        nc.vector.tensor_scalar_min(rivq, rivq, 1e24)
                        rnormq = persist.tile([8, N], BF16, tag="rnormq")
                        nc.scalar.activation(
                            out=rnormq, in_=rivq, func=AF.Sqrt, bias=0.0,
                            scale=scalesq[:, 0:1])
                        rnq_d = dram.tile([8, N], BF16, tag="rnq_d")
                        nc.sync.dma_start(out=rnq_d, in_=rnormq)
                        rnb = persist.tile([128, CB, N], BF16, tag="rnb")
                        for f in range(CB):
                            for phi in range(2):
                                nc.sync.dma_start(
                                    out=rnb[64 * phi:64 * (phi + 1), f, :],
                                    in_=rnq_d[2 * f + phi:2 * f + phi + 1, :]
                                    .to_broadcast((64, N)))
                        for f in range(CB):
                            nc.vector.tensor_mul(
                                qsT[:, f, :], qkT[:, f, :], rnb[:, f, :])
                    if fb == FB - 1:
                        rivk = persist.tile([8, N], F32, tag="rivk")
                        nc.vector.reciprocal_approx_fast(rivk, ssk)
                        nc.vector.tensor_scalar_min(rivk, rivk, 1e24)
                        rnormk = persist.tile([8, N], BF16, tag="rnormk")
                        nc.scalar.activation(
                            out=rnormk, in_=rivk, func=AF.Sqrt, bias=0.0,
                            scale=1.0)

            if stage < 2:
                _debug_out(nc, osp, qkT, outT_d)
            # ---------------- B: k-norm transposes + query-embedding ------
            if stage >= 2:
                # rkT[tok-part, kt, h]: transpose of k rows of rnorm (f32),
                # qekn = (qkT_k.T @ qesbd) * rkT  (query-embedding bias)
                for kt in range(NT):
                    pt = ps_av.tile([128, 8], BF16, tag="av")
                    nc.tensor.transpose(
                        pt, rnormk[0:8, kt * 128:(kt + 1) * 128],
                        ident[0:8, 0:8])
                    nc.vector.tensor_copy(rkT[:, kt, :], pt)
                    pq = ps_av.tile([128, H], F32, tag="av")
                    for cb in range(CB):
                        nc.tensor.matmul(
                            pq,
                            qkT[:, CB + cb, kt * 128:(kt + 1) * 128],
                            qesbd[:, cb, :],
                            start=(cb == 0), stop=(cb == CB - 1),
                        )
                    nc.vector.tensor_mul(qekn[:, kt, :], pq, rkT[:, kt, :])


            if stage == 2:
                _debug_out(nc, osp, qsT, outT_d)
            # ---------------- D: attention ----------------
            outhT = persist.tile([128, CB, N], BF16, tag="outhT")
            pending_div = []
            prev_tail = None

            def flush_div():
                # division-mul deferred a full head: by now the rrb broadcast
                # DMA has landed, so this never stalls the DVE queue head
                ph, prrb, ppav = pending_div.pop(0)
                php, ppo = ph // 2, (ph % 2) * 64
                nc.vector.tensor_mul(
                    outhT[:, php, :][ppo:ppo + 64], ppav[64:128, :], prrb)

            for h in range(H if stage >= 3 else 0):
                hp, po = h // 2, (h % 2) * 64
                expb = expb_tiles[h]
                if h + 3 < H:
                    nxt = expbp.tile([128, NT, N], BF16, tag="expb",
                                     name=f"expb{h + 3}")
                    expb_tiles.append(nxt)
                    nc.sync.dma_start(
                        out=nxt,
                        in_=expbT_d[h + 3].rearrange("p (kt q) -> p kt q", kt=NT))
                pav = ps_av.tile([128, N], F32, tag="av", name=f"pav{h}")

                def av_mm(xpav, xh, kt, eT):
                    for qb in range(2):
                        nc.tensor.matmul(
                            xpav[:, qb * 512:(qb + 1) * 512],
                            v_sb[:, kt, xh, :],
                            eT[:, qb * 512:(qb + 1) * 512],
                            start=(kt == 0), stop=(kt == NT - 1),
                        )

                def finish_head(tail):
                    # previous head's tail AVs + division prep, emitted inside
                    # this head's dense QK stream so nothing bubbles the PE
                    th, tpav, teTs = tail
                    av_mm(tpav, th, NT - 2, teTs[NT - 2])
                    av_mm(tpav, th, NT - 1, teTs[NT - 1])
                    rrec = rrp.tile([1, N], F32, tag="rrec")
                    nc.vector.reciprocal_approx_fast(rrec, tpav[0:1, :])
                    rrec_d = dram.tile([1, N], F32, tag="rrec_d")
                    nc.gpsimd.dma_start(out=rrec_d, in_=rrec)
                    rrb = rrp.tile([64, N], F32, tag="rrb")
                    nc.gpsimd.dma_start(
                        out=rrb, in_=rrec_d[0:1, :].to_broadcast((64, N)))
                    pending_div.append((th, rrb, tpav))
                    if len(pending_div) > 1:
                        flush_div()

                eTs = []
                for kt in range(NT):
                    pss = ps_qk.tile([128, N], F32, tag="qk")
                    for qb in range(2):
                        nc.tensor.matmul(
                            pss[:, qb * 512:(qb + 1) * 512],
                            qkT[:, CB + hp, kt * 128:(kt + 1) * 128][po:po + 64],
                            qsT[:, hp, qb * 512:(qb + 1) * 512][po:po + 64],
                            start=True, stop=True,
                        )
                    etmp = etp.tile([128, N], BF16, tag="etmp")
                    nc.scalar.activation(
                        out=etmp, in_=pss, func=AF.Exp,
                        bias=qekn[:, kt, h:h + 1], scale=rkT[:, kt, h:h + 1])
                    eT = eTp.tile([128, N], BF16, tag="eT", name=f"eT{h}_{kt}")
                    mul_eng = nc.vector
                    mul_eng.tensor_mul(eT, etmp, expb[:, kt, :])
                    eTs.append(eT)
                    if kt == 1 and prev_tail is not None:
                        finish_head(prev_tail)
                    # AV trails the mul by 2 kt so a slow mul cannot bubble
                    # the in-order PE queue
                    if kt >= 2:
                        av_mm(pav, h, kt - 2, eTs[kt - 2])
                prev_tail = (h, pav, eTs)
            if stage >= 3:
                finish_head(prev_tail)
                flush_div()

            if stage == 3:
                _debug_out(nc, osp, outhT, outT_d)
            # ---------------- E: output projection (c-major) ----------------
            # fb0-2 partials of the first two channel groups run during the
            # last head's division round-trip (keeps the PE warm into E)
            if stage >= 4:
                psE = {}

                def e_mms(cc, fbs):
                    for fb in fbs:
                        for th in range(2):
                            nc.tensor.matmul(
                                psE[cc][:, th * 512:(th + 1) * 512],
                                projwT[:, fb, cc * 128:(cc + 1) * 128],
                                outhT[:, fb, th * 512:(th + 1) * 512],
                                start=(fb == 0), stop=(fb == CB - 1),
                            )

                def e_finish(cc):
                    osb = osp.tile([128, N], F32, tag="osb")
                    nc.vector.tensor_scalar_add(osb, psE[cc], projb[:, cc:cc + 1])
                    for half in range(2):
                        eng = nc.scalar if (2 * cc + half) % 2 == 0 else nc.sync
                        eng.dma_start(
                            out=outT_d[cc * 128:(cc + 1) * 128,
                                       half * 512:(half + 1) * 512],
                            in_=osb[:, half * 512:(half + 1) * 512])

                for cc in range(2):
                    psE[cc] = ps_qk.tile([128, N], F32, tag="qk", name=f"psE{cc}")
                    e_mms(cc, [0, 1, 2])
                for cc in range(2):
                    e_mms(cc, [3])
                    e_finish(cc)
                for cc in range(2, CB):
                    psE[cc] = ps_qk.tile([128, N], F32, tag="qk", name=f"psE{cc}")
                    e_mms(cc, [0, 1, 2, 3])
                    e_finish(cc)

    nc.compile()
    return nc


def _debug_out(nc, osp, dbg, outT_d):
    for cc in range(CB):
        osb = osp.tile([128, N], F32, tag="osb")
        nc.vector.tensor_copy(osb, dbg[:, cc, :])
        nc.scalar.dma_start(out=outT_d[cc * 128:(cc + 1) * 128, :], in_=osb)


def _host_prep(inputs):
    """Host-side layout/scalar prep. Returns per-core input maps."""
    x = np.asarray(inputs["x"], dtype=np.float32)
    qkv_w = np.asarray(inputs["qkv_w"], dtype=np.float32)
    qkv_b = np.asarray(inputs["qkv_b"], dtype=np.float32)
    proj_w = np.asarray(inputs["proj_w"], dtype=np.float32)
    proj_b = np.asarray(inputs["proj_b"], dtype=np.float32)
    temp = np.asarray(inputs["temperature"], dtype=np.float32).reshape(H)
    qe = np.asarray(inputs["query_embedding"], dtype=np.float32).reshape(H, HD)
    tab = np.asarray(inputs["relative_coords_table"], dtype=np.float32)
    idx = np.asarray(inputs["relative_pos_index"])
    f1w = np.asarray(inputs["cpb_fc1_w"], dtype=np.float32)
    f1b = np.asarray(inputs["cpb_fc1_b"], dtype=np.float32)
    f2w = np.asarray(inputs["cpb_fc2_w"], dtype=np.float32)
    f2b = np.asarray(inputs["cpb_fc2_b"], dtype=np.float32)
    sls = np.asarray(inputs["seq_length_scale"], dtype=np.float32)

    # softplus(temperature) * seq_length_scale
    scale = (np.logaddexp(0.0, temp) * sls[0]).astype(np.float32)

    # continuous position bias table -> gathered, transposed, exponentiated
    hidden = np.maximum(tab @ f1w.T + f1b, 0.0)
    bias_tab = (hidden @ f2w.T + f2b).astype(np.float32)      # (T, H)
    bias = bias_tab[idx]                                       # (q, k, H)
    expbT = np.exp(np.transpose(bias, (2, 1, 0)))              # (H, k, q)
    # tile so each SBUF partition reads one contiguous 16KB run:
    # [H, kt, p, q] -> [H, p, kt, q]  (k = kt*128 + p)
    expbT = expbT.reshape(H, NT, 128, N).transpose(0, 2, 1, 3)
    expbT = np.ascontiguousarray(expbT).astype(NB_BF16).reshape(H, 128, NT * N)

    wqkT = np.ascontiguousarray(qkv_w[:2 * C].T).astype(NB_BF16)   # (cin, 1024)
    wvT = np.ascontiguousarray(qkv_w[2 * C:].T).astype(NB_BF16)    # (cin, 512)
    projwT = np.ascontiguousarray(proj_w.T).astype(NB_BF16)        # (cin, 512)
    qkb = qkv_b[:2 * C].reshape(2 * C, 1).copy()
    vb = qkv_b[2 * C:]
    # fold v-bias through the projection:  (o + vb) @ W.T + b = o@W.T + b'
    projb = (proj_b + vb @ proj_w.T).reshape(C, 1).astype(np.float32)
    qesbd = np.zeros((C, H), dtype=np.float32)
    for h in range(H):
        qesbd[h * HD:(h + 1) * HD, h] = qe[h] * scale[h]
    qesbd = qesbd.astype(NB_BF16)
    # q-heads get scale_h^2 inside the sqrt (k path uses scale=1.0)
    scalesq = (scale * scale).reshape(8, 1).astype(np.float32)

    # scatter-stationaries for per-head sumsq: [fb][128, 8]
    ssum16 = np.zeros((FB, 128, 8), dtype=NB_BF16)
    for f in range(FB):
        j = f % CB
        ssum16[f, 0:64, 2 * j] = 1.0
        ssum16[f, 64:128, 2 * j + 1] = 1.0
    ssum16 = np.ascontiguousarray(ssum16).reshape(2 * C, 8)
    ident = np.eye(128, dtype=NB_BF16)

    shared = dict(
        wqkT=wqkT, wvT=wvT, qkb=qkb, qesbd=qesbd, scalesq=scalesq,
        projwT=projwT, projb=projb, ssum16=ssum16, ident=ident, expbT=expbT,
    )
    in_maps = []
    for b in range(B):
        m = dict(shared)
        m["xT"] = np.ascontiguousarray(x[b].T).astype(NB_BF16)
        in_maps.append(m)
    return in_maps


def _assemble(res):
    """Gather per-core c-major outputs into the full (B, N, C) result."""
    return np.stack(
        [np.ascontiguousarray(res.results[b]["outT"].T) for b in range(B)],
        axis=0).astype(np.float32)


def get_nc():
    key = "nc"
    if key not in _CACHE:
        stage = int(os.environ.get("BASS_STAGE", "4"))
        _CACHE[key] = _build(stage)
    return _CACHE[key]


def kernel(**inputs) -> np.ndarray:
    nc = get_nc()
    in_maps = _host_prep(inputs)
    res = run_bass_kernel_spmd(nc, in_maps, core_ids=list(range(B)))
    return _assemble(res)
